# revision 3
# baseline (speedup 1.0000x reference)
"""Trainium2 Bass kernel for nn_CompressedInteractionNet_31997506355236.

Reference math (per batch b, channel k, dim d; m == H == 64, D == 16, vk == 16):
    A[b,d,k,v] = sum_i x_0[b,i,d] * Vm[k,i,v]
    B[b,d,k,v] = sum_j xflat[b,d,j] * Vh[k,v,j]   (xflat = x_0 reshaped [B,D,H])
    out[b,k,d] = sum_v A[b,d,k,v] * B[b,d,k,v]

Sharding: batch x channels = 4 x 2 over 8 cores (32 batches, 32 channels per
core). All inputs are cast to bf16 on the host (tolerance 2e-2 >> bf16 error)
to halve DMA bytes; per-core inputs are packed into two fully-contiguous
[64, 1024] bf16 tensors so each load is one 2KB-per-partition DMA on a
hardware queue.

Device layout puts kv (= k*16+v, 512 per core) on PSUM partitions in 4 blocks
of 128 and bd (= 32 batches x 16 dims = 512) on the free axis:
    A_blk  = vmf_blk.T @ x0t     [128 kv, 512 bd]   (PE, bf16)
    B_blk  = vhf_blk.T @ xhrt    [128 kv, 512 bd]
    b_sb   = copy(B_blk)                            (ACT; DVE allows <=1 PSUM in)
    p_sb   = A_blk * b_sb -> bf16                   (DVE)
    out   += E_blk.T @ p_sb                         (PE; E is block-ones, this
                                                     does the v-reduction AND
                                                     accumulates all 4 blocks
                                                     into one [32, 512] bank)
The result lands as [k, (b,d)]: one contiguous 2KB-per-partition output DMA.
Dummy warmup matmuls run during the input-DMA wait so the PE clock is ramped
when real matmuls start.
"""

import numpy as np
import ml_dtypes

import concourse.bass as bass
import concourse.tile as tile
from concourse import bacc, mybir
from concourse.bass_utils import run_bass_kernel_spmd

# Problem constants (hardcoded; kernel must be self-contained).
B, M, D = 128, 64, 16
HK, VK = 64, 16
H = 64
NCORES = 8
SB, SK = 4, 2             # batch shards x channel shards
BL = B // SB              # batches per core = 32
BD = BL * D               # free rows per core = 512
KL = HK // SK             # channels per core = 32
KVL = KL * VK             # kv per core = 512
NBLK = KVL // 128         # kv blocks of 128 partitions = 4
NWARM = 4                 # PE warmup matmuls during the input-DMA wait
F32 = mybir.dt.float32
BF16 = mybir.dt.bfloat16
BF16_NP = ml_dtypes.bfloat16

_CACHE = {}


def build_bass():
    nc = bacc.Bacc("TRN2", target_bir_lowering=False, debug=False,
                   num_devices=NCORES, enable_partition_id=False,
                   monotonic_sem_count=0)

    xc_d = nc.dram_tensor("xc", [M, 2 * BD], BF16, kind="ExternalInput")
    vc_d = nc.dram_tensor("vc", [M, 2 * KVL], BF16, kind="ExternalInput")
    ee_d = nc.dram_tensor("ee", [128, NBLK * KL], BF16, kind="ExternalInput")
    out = nc.dram_tensor("out", [KL, BD], F32, kind="ExternalOutput")

    with tile.TileContext(nc) as tc:
        with (
            tc.tile_pool(name="w", bufs=1) as w,
            tc.tile_pool(name="bs", bufs=2) as bs,
            tc.tile_pool(name="ps", bufs=3) as ps,
            tc.tile_pool(name="pa", bufs=3, space="PSUM") as pa,
            tc.tile_pool(name="pb", bufs=2, space="PSUM") as pb,
            tc.tile_pool(name="po", bufs=1, space="PSUM") as po,
            tc.tile_pool(name="pw", bufs=1, space="PSUM") as pw,
        ):
            # Input loads on the two fastest hardware queues; E + the warmup
            # dummy on the vector queue (gpsimd's software queue is slow and
            # its first use would pull in a library load).
            xc = w.tile([M, 2 * BD], BF16)
            nc.sync.dma_start(xc[:], xc_d.ap())
            vc = w.tile([M, 2 * KVL], BF16)
            nc.scalar.dma_start(vc[:], vc_d.ap())
            ee = w.tile([128, NBLK * KL], BF16)
            nc.sync.dma_start(ee[:], ee_d.ap())
            dum = w.tile([M, BD], BF16)
            nc.vector.memset(dum[:], 0.0)

            # PE p-state warmup on dummy data while the input DMAs land.
            scr = pw.tile([128, BD], F32)
            for i in range(NWARM):
                nc.tensor.matmul(scr[:], dum[:, 0:128], dum[:],
                                 start=(i == 0), stop=(i == NWARM - 1))

            psum_o = po.tile([KL, BD], F32)
            p_tiles = []

            def emit_red(c):
                nc.tensor.matmul(psum_o[:], ee[:, KL * c:KL * (c + 1)],
                                 p_tiles[c][:],
                                 start=(c == 0), stop=(c == NBLK - 1))

            for c in range(NBLK):
                lo = 128 * c
                pb_t = pb.tile([128, BD], F32, tag="b")
                nc.tensor.matmul(pb_t[:], vc[:, KVL + lo:KVL + lo + 128],
                                 xc[:, BD:2 * BD], start=True, stop=True)
                pa_t = pa.tile([128, BD], F32, tag="a")
                nc.tensor.matmul(pa_t[:], vc[:, lo:lo + 128],
                                 xc[:, 0:BD], start=True, stop=True)

                b_sb = bs.tile([128, BD], F32, tag="bs")
                nc.scalar.copy(b_sb[:], pb_t[:])
                p_sb = ps.tile([128, BD], BF16, tag="ps")
                nc.vector.tensor_mul(out=p_sb[:], in0=pa_t[:], in1=b_sb[:])
                p_tiles.append(p_sb)
                if c >= 1:
                    emit_red(c - 1)
            emit_red(NBLK - 1)

            o_sb = w.tile([KL, BD], F32)
            nc.scalar.copy(o_sb[:], psum_o[:])
            nc.sync.dma_start(out.ap(), o_sb[:])

    nc.compile()
    return nc


def _host_shards(x_0, Vm, Vh):
    """Per-core input tensors, all contiguous bf16."""
    x_0 = np.ascontiguousarray(np.asarray(x_0), dtype=np.float32)
    vm = np.asarray(Vm)[:, 0].astype(np.float32)     # [HK, M, VK]
    vh = np.asarray(Vh)[:, 0].astype(np.float32)     # [HK, VK, H]

    vmf = vm.transpose(1, 0, 2).reshape(M, HK * VK)  # [i, (k,v)]
    vhf = vh.transpose(2, 0, 1).reshape(H, HK * VK)  # [j, (k,v)]

    # E[p, KL*c + k] = 1 iff k == 8c + p//16 : contracting p over a kv block
    # sums v and routes to output row k.
    ee = np.zeros((128, NBLK * KL), dtype=np.float32)
    for c in range(NBLK):
        for p in range(128):
            ee[p, KL * c + 8 * c + p // VK] = 1.0
    ee = ee.astype(BF16_NP)

    in_maps = []
    for core in range(NCORES):
        cb, ck = divmod(core, SK)
        shard = x_0[BL * cb:BL * (cb + 1)]                    # [BL, M, D]
        x0t = shard.transpose(1, 0, 2).reshape(M, BD)         # [i, (b,d)]
        xhrt = shard.reshape(BL, D, H).transpose(2, 0, 1).reshape(H, BD)
        xc = np.concatenate([x0t, xhrt], axis=1).astype(BF16_NP)
        ks = slice(KVL * ck, KVL * (ck + 1))
        vcc = np.concatenate([vmf[:, ks], vhf[:, ks]], axis=1).astype(BF16_NP)
        in_maps.append({
            "xc": np.ascontiguousarray(xc),
            "vc": np.ascontiguousarray(vcc),
            "ee": ee,
        })
    return in_maps


def run(x_0, x_h, Vm, Vh, **spmd_kwargs):
    in_maps = _host_shards(x_0, Vm, Vh)

    if "nc" not in _CACHE:
        _CACHE["nc"] = build_bass()
    nc = _CACHE["nc"]

    res = run_bass_kernel_spmd(nc, in_maps, core_ids=list(range(NCORES)),
                               **spmd_kwargs)
    # Unshard: per-core out is [k_loc, (b_loc, d)] -> [KL, BL, D]
    full = np.empty((B, HK, D), dtype=np.float32)
    for core in range(NCORES):
        cb, ck = divmod(core, SK)
        o = res.results[core]["out"].reshape(KL, BL, D).transpose(1, 0, 2)
        full[BL * cb:BL * (cb + 1), KL * ck:KL * (ck + 1), :] = o
    return full, res


def kernel(x_0, x_h, Vm, Vh):
    return run(x_0, x_h, Vm, Vh)[0]


if __name__ == "__main__":
    rng = np.random.default_rng(0)
    x_0 = rng.standard_normal((B, M, D)).astype(np.float32)
    x_h = rng.standard_normal((B, H, D)).astype(np.float32)
    Vm = (0.01 * rng.standard_normal((HK, 1, M, VK))).astype(np.float32)
    Vh = (0.01 * rng.standard_normal((HK, 1, VK, H))).astype(np.float32)
    got = kernel(x_0, x_h, Vm, Vh)

    x0r = np.transpose(x_0, (0, 2, 1))
    xhr = x_0.reshape(B, D, H)
    a = np.einsum("bdi,kiv->bkdv", x0r, Vm[:, 0])
    bb = np.einsum("bdj,kvj->bkdv", xhr, Vh[:, 0])
    want = np.einsum("bkdv,bkdv->bkd", a, bb)
    err = np.abs(got - want).max() / np.abs(want).max()
    print("rel err:", err)


# revision 6
# speedup vs baseline: 1.0530x; 1.0530x over previous
"""Trainium2 Bass kernel for nn_CompressedInteractionNet_31997506355236.

Reference math (per batch b, channel k, dim d; m == H == 64, D == 16, vk == 16):
    A[b,d,k,v] = sum_i x_0[b,i,d] * Vm[k,i,v]
    B[b,d,k,v] = sum_j xflat[b,d,j] * Vh[k,v,j]   (xflat = x_0 reshaped [B,D,H])
    out[b,k,d] = sum_v A[b,d,k,v] * B[b,d,k,v]

Sharding: batch x channels = 4 x 2 over 8 cores (32 batches, 32 channels per
core). All inputs are cast to bf16 on the host (tolerance 2e-2 >> bf16 error)
and packed into ONE fully-contiguous [64, 2048] bf16 tensor per core
([x0t | xhrt | vmf | vhf]), so the entire input load is a single 64-descriptor
DMA on the sync hardware queue (descriptor generation on the shared DGE engine
was the input-latency bottleneck).

Device: bd = (batch,dim) rows on PSUM partitions in 4 chunks of 128, kv on the
free axis:
    B_c  = xhrt_c.T @ vhf       [128 bd, 512 kv]   (PE, bf16)
    A_c  = x0t_c.T @ vmf        [128 bd, 512 kv]
    p    = A_c * B_c -> fp16                       (DVE, both inputs PSUM)
    o[:, 32c:32c+32] = reduce_v p                  (DVE, fp16, 2x/4x mode)
Output is one [128, 256B] fp16 DMA; host unshards/upcasts. Dummy warmup
matmuls run during the input-DMA wait to ramp the PE clock.

Set TWO_PSUM_MUL=False to fall back to the ACT-copy variant if hardware
disallows two PSUM operands on the DVE.
"""

import numpy as np
import ml_dtypes

import concourse.bass as bass
import concourse.tile as tile
from concourse import bacc, mybir
from concourse.bass_utils import run_bass_kernel_spmd

# Problem constants (hardcoded; kernel must be self-contained).
B, M, D = 128, 64, 16
HK, VK = 64, 16
H = 64
NCORES = 8
SB, SK = 4, 2             # batch shards x channel shards
BL = B // SB              # batches per core = 32
BD = BL * D               # bd rows per core = 512
KL = HK // SK             # channels per core = 32
KVL = KL * VK             # kv per core = 512
NCH = BD // 128           # 128-row bd chunks = 4
NWARM = 6                 # PE warmup matmuls during the input-DMA wait
TWO_PSUM_MUL = False      # HW verifier: DVE allows only one PSUM operand
F32 = mybir.dt.float32
BF16 = mybir.dt.bfloat16
FP16 = mybir.dt.float16
BF16_NP = ml_dtypes.bfloat16

_CACHE = {}


def build_bass():
    nc = bacc.Bacc("TRN2", target_bir_lowering=False, debug=False,
                   num_devices=NCORES, enable_partition_id=False,
                   monotonic_sem_count=0)

    # [x0t | xhrt | vmf | vhf], each [64, 512] bf16
    ain_d = nc.dram_tensor("ain", [M, 4 * 512], BF16, kind="ExternalInput")
    out = nc.dram_tensor("out", [128, NCH * KL], FP16, kind="ExternalOutput")

    with tile.TileContext(nc) as tc:
        with (
            tc.tile_pool(name="w", bufs=1) as w,
            tc.tile_pool(name="bs", bufs=2) as bs,
            tc.tile_pool(name="ps", bufs=3) as ps,
            tc.tile_pool(name="pa", bufs=3, space="PSUM") as pa,
            tc.tile_pool(name="pb", bufs=3, space="PSUM") as pb,
            tc.tile_pool(name="pw", bufs=1, space="PSUM") as pw,
        ):
            ain = w.tile([M, 4 * 512], BF16)
            nc.sync.dma_start(ain[:], ain_d.ap())
            dum = w.tile([M, 512], BF16)
            nc.vector.memset(dum[:], 0.0)

            # PE p-state warmup on dummy data while the input DMA lands.
            scr = pw.tile([128, 512], F32)
            for i in range(NWARM):
                nc.tensor.matmul(scr[:], dum[:, 0:128], dum[:],
                                 start=(i == 0), stop=(i == NWARM - 1))

            o_all = w.tile([128, NCH, KL], FP16)
            for c in range(NCH):
                lo = 128 * c
                pb_t = pb.tile([128, KVL], F32, tag="b")
                nc.tensor.matmul(pb_t[:], ain[:, 512 + lo:512 + lo + 128],
                                 ain[:, 1536:2048], start=True, stop=True)
                pa_t = pa.tile([128, KVL], F32, tag="a")
                nc.tensor.matmul(pa_t[:], ain[:, lo:lo + 128],
                                 ain[:, 1024:1536], start=True, stop=True)

                p_sb = ps.tile([128, KL, VK], FP16, tag="p")
                if TWO_PSUM_MUL:
                    nc.vector.tensor_mul(out=p_sb.rearrange("p k v -> p (k v)"),
                                         in0=pa_t[:], in1=pb_t[:])
                else:
                    b_sb = bs.tile([128, KVL], F32, tag="bs")
                    nc.scalar.copy(b_sb[:], pb_t[:])
                    nc.vector.tensor_mul(out=p_sb.rearrange("p k v -> p (k v)"),
                                         in0=pa_t[:], in1=b_sb[:])
                # fp16 accumulation over 16 terms: error ~1e-3, tolerance 2e-2
                with nc.allow_low_precision(reason="fp16 reduce over v=16"):
                    nc.vector.tensor_reduce(out=o_all[:, c, :], in_=p_sb[:],
                                            axis=mybir.AxisListType.X,
                                            op=mybir.AluOpType.add)

            nc.sync.dma_start(out.ap(), o_all.rearrange("p c k -> p (c k)"))

    nc.compile()
    return nc


def _host_shards(x_0, Vm, Vh):
    """Per-core packed input tensors, contiguous bf16."""
    x_0 = np.ascontiguousarray(np.asarray(x_0), dtype=np.float32)
    vm = np.asarray(Vm)[:, 0].astype(np.float32)     # [HK, M, VK]
    vh = np.asarray(Vh)[:, 0].astype(np.float32)     # [HK, VK, H]

    vmf = vm.transpose(1, 0, 2).reshape(M, HK * VK)  # [i, (k,v)]
    vhf = vh.transpose(2, 0, 1).reshape(H, HK * VK)  # [j, (k,v)]

    in_maps = []
    for core in range(NCORES):
        cb, ck = divmod(core, SK)
        shard = x_0[BL * cb:BL * (cb + 1)]                    # [BL, M, D]
        x0t = shard.transpose(1, 0, 2).reshape(M, BD)         # [i, (b,d)]
        xhrt = shard.reshape(BL, D, H).transpose(2, 0, 1).reshape(H, BD)
        ks = slice(KVL * ck, KVL * (ck + 1))
        ain = np.concatenate([x0t, xhrt, vmf[:, ks], vhf[:, ks]],
                             axis=1).astype(BF16_NP)
        in_maps.append({"ain": np.ascontiguousarray(ain)})
    return in_maps


def run(x_0, x_h, Vm, Vh, **spmd_kwargs):
    in_maps = _host_shards(x_0, Vm, Vh)

    if "nc" not in _CACHE:
        _CACHE["nc"] = build_bass()
    nc = _CACHE["nc"]

    res = run_bass_kernel_spmd(nc, in_maps, core_ids=list(range(NCORES)),
                               **spmd_kwargs)
    # Unshard: per-core out is [p, (c, k)] with bd = 128c + p -> [BL, D, KL]
    full = np.empty((B, HK, D), dtype=np.float32)
    for core in range(NCORES):
        cb, ck = divmod(core, SK)
        o = res.results[core]["out"].astype(np.float32).reshape(128, NCH, KL)
        o = o.transpose(1, 0, 2).reshape(BL, D, KL).transpose(0, 2, 1)
        full[BL * cb:BL * (cb + 1), KL * ck:KL * (ck + 1), :] = o
    return full, res


def kernel(x_0, x_h, Vm, Vh):
    return run(x_0, x_h, Vm, Vh)[0]


if __name__ == "__main__":
    rng = np.random.default_rng(0)
    x_0 = rng.standard_normal((B, M, D)).astype(np.float32)
    x_h = rng.standard_normal((B, H, D)).astype(np.float32)
    Vm = (0.01 * rng.standard_normal((HK, 1, M, VK))).astype(np.float32)
    Vh = (0.01 * rng.standard_normal((HK, 1, VK, H))).astype(np.float32)
    got = kernel(x_0, x_h, Vm, Vh)

    x0r = np.transpose(x_0, (0, 2, 1))
    xhr = x_0.reshape(B, D, H)
    a = np.einsum("bdi,kiv->bkdv", x0r, Vm[:, 0])
    bb = np.einsum("bdj,kvj->bkdv", xhr, Vh[:, 0])
    want = np.einsum("bkdv,bkdv->bkd", a, bb)
    err = np.abs(got - want).max() / np.abs(want).max()
    print("rel err:", err)


# revision 8
# speedup vs baseline: 1.1350x; 1.0778x over previous
"""Trainium2 Bass kernel for nn_CompressedInteractionNet_31997506355236.

Reference math (per batch b, channel k, dim d; m == H == 64, D == 16, vk == 16):
    A[b,d,k,v] = sum_i x_0[b,i,d] * Vm[k,i,v]
    B[b,d,k,v] = sum_j xflat[b,d,j] * Vh[k,v,j]   (xflat = x_0 reshaped [B,D,H])
    out[b,k,d] = sum_v A[b,d,k,v] * B[b,d,k,v]

Sharding: batch x channels = 4 x 2 over 8 cores (32 batches, 32 channels per
core). Inputs are cast to bf16 (tolerance 2e-2 >> bf16 error).

PE array packing: the contraction dim is 64, so the 128x128 PE runs in 64-row
tiling mode with two independent tiles (T0 = SBUF partitions 0-63, T8 =
64-127). The B-side operands (vhf, xhrt) live on partitions 0-63 and the
A-side (vmf, x0t) on 64-127, so each chunk's A and B matmuls execute
CONCURRENTLY on the two tiles (tile_position auto-derives from the operand
base partitions). Each side loads with its own 64-descriptor DMA: B-side on
the sync hardware queue, A-side on the gpsimd software-DGE queue so the
descriptor generations don't serialize on the shared DGE engine.

Per 128-row bd chunk:
    B_c  = xhrt_c.T @ vhf   [128 bd, 512 kv]  (PE T0)   } concurrent
    A_c  = x0t_c.T @ vmf    [128 bd, 512 kv]  (PE T8)   }
    b_sb = copy(B_c)                          (ACT; DVE allows 1 PSUM operand)
    p    = A_c * b_sb                         (DVE, fp32)
    o[:, c, :] = sum_v p: chunks 0,1 via a 4-stage GPSIMD add-fold tree,
    chunks 2,3 via DVE tensor_reduce (DVE is the bottleneck engine; GPSIMD
    absorbs the early reductions, DVE keeps only the tail ones).
Output is one [128, 512B] fp32 DMA; the host unshards. Dummy warmup matmuls
keep the PE clock up during the input-DMA wait.
"""

import numpy as np
import ml_dtypes

import concourse.bass as bass
import concourse.tile as tile
from concourse import bacc, mybir
from concourse.bass_utils import run_bass_kernel_spmd

# Problem constants (hardcoded; kernel must be self-contained).
B, M, D = 128, 64, 16
HK, VK = 64, 16
H = 64
NCORES = 8
SB, SK = 4, 2             # batch shards x channel shards
BL = B // SB              # batches per core = 32
BD = BL * D               # bd rows per core = 512
KL = HK // SK             # channels per core = 32
KVL = KL * VK             # kv per core = 512
NCH = BD // 128           # 128-row bd chunks = 4
NWARM = 10                # PE warmup matmuls during the input-DMA wait
F32 = mybir.dt.float32
BF16 = mybir.dt.bfloat16
BF16_NP = ml_dtypes.bfloat16

_CACHE = {}


def build_bass():
    nc = bacc.Bacc("TRN2", target_bir_lowering=False, debug=False,
                   num_devices=NCORES, enable_partition_id=False,
                   monotonic_sem_count=0)

    # B-side [vhf | xhrt] -> SBUF partitions 0-63; A-side [vmf | x0t] -> 64-127
    blo_d = nc.dram_tensor("blo", [M, 2 * 512], BF16, kind="ExternalInput")
    ahi_d = nc.dram_tensor("ahi", [M, 2 * 512], BF16, kind="ExternalInput")
    out = nc.dram_tensor("out", [128, NCH * KL], F32, kind="ExternalOutput")

    with tile.TileContext(nc) as tc:
        with (
            tc.tile_pool(name="w", bufs=1) as w,
            tc.tile_pool(name="bs", bufs=2) as bs,
            tc.tile_pool(name="ps", bufs=3) as ps,
            tc.tile_pool(name="gt", bufs=2) as gt,
            tc.tile_pool(name="pa", bufs=3, space="PSUM") as pa,
            tc.tile_pool(name="pb", bufs=3, space="PSUM") as pb,
            tc.tile_pool(name="pw", bufs=1, space="PSUM") as pw,
        ):
            xin = w.tile([128, 1024], BF16)
            nc.sync.dma_start(xin[0:64, :], blo_d.ap())
            nc.gpsimd.dma_start(xin[64:128, :], ahi_d.ap())
            dum = w.tile([M, 256], BF16)
            nc.vector.memset(dum[:], 0.0)

            vhf = xin[0:64, 0:512]
            vmf = xin[64:128, 0:512]

            # PE p-state warmup on dummy data while the input DMAs land.
            scr = pw.tile([128, 256], F32)
            for i in range(NWARM):
                nc.tensor.matmul(scr[:], dum[:, 0:128], dum[:],
                                 start=(i == 0), stop=(i == NWARM - 1))

            o_all = w.tile([128, NCH, KL], F32)

            def gps_fold(p_sb, c):
                """4-stage add tree over v=16 on GPSIMD."""
                t8 = gt.tile([128, KL, 8], F32, tag="t8")
                nc.gpsimd.tensor_tensor(t8[:], p_sb[:, :, 0:8], p_sb[:, :, 8:16],
                                        mybir.AluOpType.add)
                t4 = gt.tile([128, KL, 4], F32, tag="t4")
                nc.gpsimd.tensor_tensor(t4[:], t8[:, :, 0:4], t8[:, :, 4:8],
                                        mybir.AluOpType.add)
                t2 = gt.tile([128, KL, 2], F32, tag="t2")
                nc.gpsimd.tensor_tensor(t2[:], t4[:, :, 0:2], t4[:, :, 2:4],
                                        mybir.AluOpType.add)
                nc.gpsimd.tensor_tensor(o_all[:, c, :],
                                        t2[:, :, 0:1].rearrange("p k v -> p (k v)"),
                                        t2[:, :, 1:2].rearrange("p k v -> p (k v)"),
                                        mybir.AluOpType.add)

            for c in range(NCH):
                lo = 512 + 128 * c
                pb_t = pb.tile([128, KVL], F32, tag="b")
                nc.tensor.matmul(pb_t[:], xin[0:64, lo:lo + 128], vhf,
                                 start=True, stop=True)
                pa_t = pa.tile([128, KVL], F32, tag="a")
                nc.tensor.matmul(pa_t[:], xin[64:128, lo:lo + 128], vmf,
                                 start=True, stop=True)

                b_sb = bs.tile([128, KVL], F32, tag="bs")
                nc.scalar.copy(b_sb[:], pb_t[:])
                p_sb = ps.tile([128, KL, VK], F32, tag="p")
                nc.vector.tensor_mul(out=p_sb.rearrange("p k v -> p (k v)"),
                                     in0=pa_t[:], in1=b_sb[:])
                if c < 2:
                    gps_fold(p_sb, c)
                else:
                    nc.vector.tensor_reduce(out=o_all[:, c, :], in_=p_sb[:],
                                            axis=mybir.AxisListType.X,
                                            op=mybir.AluOpType.add)

            nc.sync.dma_start(out.ap(), o_all.rearrange("p c k -> p (c k)"))

    nc.compile()
    return nc


def _host_shards(x_0, Vm, Vh):
    """Per-core packed input tensors, contiguous bf16."""
    x_0 = np.ascontiguousarray(np.asarray(x_0), dtype=np.float32)
    vm = np.asarray(Vm)[:, 0].astype(np.float32)     # [HK, M, VK]
    vh = np.asarray(Vh)[:, 0].astype(np.float32)     # [HK, VK, H]

    vmf = vm.transpose(1, 0, 2).reshape(M, HK * VK)  # [i, (k,v)]
    vhf = vh.transpose(2, 0, 1).reshape(H, HK * VK)  # [j, (k,v)]

    in_maps = []
    for core in range(NCORES):
        cb, ck = divmod(core, SK)
        shard = x_0[BL * cb:BL * (cb + 1)]                    # [BL, M, D]
        x0t = shard.transpose(1, 0, 2).reshape(M, BD)         # [i, (b,d)]
        xhrt = shard.reshape(BL, D, H).transpose(2, 0, 1).reshape(H, BD)
        ks = slice(KVL * ck, KVL * (ck + 1))
        blo = np.concatenate([vhf[:, ks], xhrt], axis=1).astype(BF16_NP)
        ahi = np.concatenate([vmf[:, ks], x0t], axis=1).astype(BF16_NP)
        in_maps.append({"blo": np.ascontiguousarray(blo),
                        "ahi": np.ascontiguousarray(ahi)})
    return in_maps


def run(x_0, x_h, Vm, Vh, **spmd_kwargs):
    in_maps = _host_shards(x_0, Vm, Vh)

    if "nc" not in _CACHE:
        _CACHE["nc"] = build_bass()
    nc = _CACHE["nc"]

    res = run_bass_kernel_spmd(nc, in_maps, core_ids=list(range(NCORES)),
                               **spmd_kwargs)
    # Unshard: per-core out is [p, (c, k)] with bd = 128c + p -> [BL, D, KL]
    full = np.empty((B, HK, D), dtype=np.float32)
    for core in range(NCORES):
        cb, ck = divmod(core, SK)
        o = res.results[core]["out"].astype(np.float32).reshape(128, NCH, KL)
        o = o.transpose(1, 0, 2).reshape(BL, D, KL).transpose(0, 2, 1)
        full[BL * cb:BL * (cb + 1), KL * ck:KL * (ck + 1), :] = o
    return full, res


def kernel(x_0, x_h, Vm, Vh):
    return run(x_0, x_h, Vm, Vh)[0]


if __name__ == "__main__":
    rng = np.random.default_rng(0)
    x_0 = rng.standard_normal((B, M, D)).astype(np.float32)
    x_h = rng.standard_normal((B, H, D)).astype(np.float32)
    Vm = (0.01 * rng.standard_normal((HK, 1, M, VK))).astype(np.float32)
    Vh = (0.01 * rng.standard_normal((HK, 1, VK, H))).astype(np.float32)
    got = kernel(x_0, x_h, Vm, Vh)

    x0r = np.transpose(x_0, (0, 2, 1))
    xhr = x_0.reshape(B, D, H)
    a = np.einsum("bdi,kiv->bkdv", x0r, Vm[:, 0])
    bb = np.einsum("bdj,kvj->bkdv", xhr, Vh[:, 0])
    want = np.einsum("bkdv,bkdv->bkd", a, bb)
    err = np.abs(got - want).max() / np.abs(want).max()
    print("rel err:", err)
